# revision 7
# baseline (speedup 1.0000x reference)
"""Multi-class 3D DICE loss on 8 Trainium2 NeuronCores.

Data-parallel over the subject (batch) axis: core b reduces subject b's
[C=4, 64, 128, 128] volumes to per-class (inter, mask_sum, out_sum) partial
sums; the host applies the ~10-flop DICE scalar tail and averages the 8
per-subject losses.

Per-core layout: each input tensor is viewed as [128, 32768] where
partition q = c*32 + p (class c in partition block [32c, 32c+32)).
Per chunk (descending sizes, 4 MiB DMAs in steady state):
  - DVE  scalar_tensor_tensor: partial sums of output*masks   (inter)
  - ACT  activation(Copy, accum_out): partial sums of output
  - ACT  activation(Copy, accum_out): partial sums of masks
  - PE   collapses partition blocks into per-class sums with one matmul
Engine budget per 8 MiB chunk-pair (~19.5 us of DMA at ~430 GB/s):
DVE one pass ~8.7 us, ACT two passes ~14.2 us — both stay ahead of the
DMA stream, so the stream never stalls on buffer reuse. The last (tiny)
chunk splits its two plain sums across ACT (x) and DVE (m) so the
post-last-byte compute tail is ~1 us. Every tail chunk gets a dedicated
buffer so all DMAs are issued with no waits and queue on the ring early.
"""

import os
import sys
from contextlib import ExitStack

import numpy as np

for _p in ("/opt/trn_rl_repo",):
    if _p not in sys.path and os.path.isdir(_p):
        sys.path.insert(0, _p)

import concourse.bass as bass  # noqa: E402
import concourse.tile as tile  # noqa: E402
from concourse import bacc, mybir  # noqa: E402
from concourse.bass_utils import run_bass_kernel_spmd  # noqa: E402

N_CORES = 8
B, C = 8, 4
SPATIAL = 64 * 128 * 128            # 1,048,576 per (subject, class)
P = 128                             # SBUF partitions = C * 32
COLS = (C * SPATIAL) // P           # 32768 elements per partition
# Descending chunk schedule: big DMAs (4 MiB) for bandwidth in the steady
# state, small chunks at the end so the post-last-byte compute tail is tiny.
CHUNKS = [8192, 8192, 8192, 4096, 2048, 1024, 512, 256, 256]
BIG_FD = 4096  # chunks >= this land in the big pools, the rest in tail pools
assert sum(CHUNKS) == COLS
NCHUNK = len(CHUNKS)
LAST = NCHUNK - 1
EPS = 1e-7
F32 = mybir.dt.float32

# Accumulator column layout ([P, 33]). SBUF accumulator words (32 B = 8
# fp32 cols) must each be written by a single engine — mixing engines
# within one word produced intermittent lost-update corruption on HW.
#   cols  0..8   inter,  DVE  (words 0-1)
#   col   15     msum of last chunk, DVE tensor_reduce (word 1, DVE-owned)
#   cols 16..23  msum of chunks 0..7, ACT (word 2)
#   cols 24..32  xsum, ACT (words 3-4)
# Cols 9..14 are memset-0 padding (DVE-owned word); the final reduces read
# exact ranges so pad columns never contaminate a result.
INTER0 = 0
MSUM_DVE = 15
MSUM0 = 16
XSUM0 = 24
ACC_COLS = 33


def _dice_body(ctx: ExitStack, tc: "tile.TileContext", out_ap, x_ap, m_ap):
    nc = tc.nc
    add = mybir.AluOpType.add
    mult = mybir.AluOpType.mult
    Copy = mybir.ActivationFunctionType.Copy

    consts = ctx.enter_context(tc.tile_pool(name="consts", bufs=1))
    xpool = ctx.enter_context(tc.tile_pool(name="xin", bufs=2))
    mpool = ctx.enter_context(tc.tile_pool(name="min", bufs=2))
    # One dedicated pool per tail (chunk, tensor): no buffer reuse, so tail
    # DMAs issue with no waits; slots are exact-sized (pool slots are all
    # max-tile-sized, so one variable-size pool would waste SBUF).
    tails = {
        (j, t): ctx.enter_context(tc.tile_pool(name=f"{t}tail{j}", bufs=1))
        for j, fd in enumerate(CHUNKS)
        if fd < BIG_FD
        for t in ("x", "m")
    }
    small = ctx.enter_context(tc.tile_pool(name="small", bufs=1))
    psum = ctx.enter_context(tc.tile_pool(name="psum", bufs=1, space="PSUM"))

    # Block indicator: ind[q, c] = 1.0 iff q // 32 == c. lhsT for the
    # partition-block -> per-class collapse.
    ind = consts.tile([P, C], F32)
    nc.vector.memset(ind[:], 0.0)
    for c in range(C):
        nc.vector.memset(ind[c * 32 : (c + 1) * 32, c : c + 1], 1.0)

    # Per-chunk partial sums (see layout above); no cross-chunk deps.
    acc = small.tile([P, ACC_COLS], F32)
    nc.vector.memset(acc[:, 9:15], 0.0)
    # Engines must write their full elementwise result somewhere; stride-0
    # broadcast dummies avoid real [P, fd] scratch tiles (HW-verified).
    dve_dummy = small.tile([P, 1], F32)
    act_dummy = small.tile([P, 1], F32)
    act_dummy2 = small.tile([P, 1], F32)

    off = 0
    for j, fd in enumerate(CHUNKS):
        big = fd >= BIG_FD
        xt = (xpool if big else tails[(j, "x")]).tile([P, fd], F32, tag="xt")
        nc.sync.dma_start(out=xt[:], in_=x_ap[:, off : off + fd])
        mt = (mpool if big else tails[(j, "m")]).tile([P, fd], F32, tag="mt")
        nc.sync.dma_start(out=mt[:], in_=m_ap[:, off : off + fd])
        off += fd

        # inter partials on DVE: out = (x*1)*m, accum = X-reduce(out).
        nc.vector.scalar_tensor_tensor(
            out=dve_dummy.broadcast_to((P, fd)),
            in0=xt[:],
            scalar=1.0,
            in1=mt[:],
            op0=mult,
            op1=mult,
            accum_out=acc[:, INTER0 + j : INTER0 + j + 1],
        )
        # x-sum on ACT (x's DMA lands before m's).
        nc.scalar.activation(
            out=act_dummy2.broadcast_to((P, fd)),
            in_=xt[:],
            func=Copy,
            accum_out=acc[:, XSUM0 + j : XSUM0 + j + 1],
        )
        if j < LAST:
            nc.scalar.activation(
                out=act_dummy.broadcast_to((P, fd)),
                in_=mt[:],
                func=Copy,
                accum_out=acc[:, MSUM0 + j : MSUM0 + j + 1],
            )
        else:
            # Last chunk: m-sum on DVE so ACT and DVE finish in parallel
            # right after the final bytes land.
            nc.vector.tensor_reduce(
                acc[:, MSUM_DVE : MSUM_DVE + 1],
                mt[:],
                axis=mybir.AxisListType.X,
                op=add,
            )

    # Partition blocks -> per-(class, chunk) sums in one matmul, then three
    # exact-range PSUM reduces -> [4, 3] class sums (inter, msum, xsum).
    # The remaining ~10-flop scalar tail runs on the host during unshard.
    ps = psum.tile([C, ACC_COLS], F32)
    nc.tensor.matmul(out=ps[:], lhsT=ind[:], rhs=acc[:], start=True, stop=True)
    sums = small.tile([C, 3], F32)
    nc.vector.tensor_reduce(
        sums[:, 0:1], ps[:, INTER0 : INTER0 + NCHUNK], axis=mybir.AxisListType.X, op=add
    )
    nc.vector.tensor_reduce(
        sums[:, 1:2], ps[:, MSUM_DVE : MSUM0 + LAST], axis=mybir.AxisListType.X, op=add
    )
    nc.vector.tensor_reduce(
        sums[:, 2:3], ps[:, XSUM0 : XSUM0 + NCHUNK], axis=mybir.AxisListType.X, op=add
    )
    nc.sync.dma_start(out=out_ap, in_=sums[:])


_CACHE: dict[str, object] = {}


def _build():
    if "nc" in _CACHE:
        return _CACHE["nc"]
    nc = bacc.Bacc("TRN2", target_bir_lowering=False, debug=False)
    x = nc.dram_tensor("x", [P, COLS], F32, kind="ExternalInput").ap()
    m = nc.dram_tensor("m", [P, COLS], F32, kind="ExternalInput").ap()
    out = nc.dram_tensor("class_sums", [C, 3], F32, kind="ExternalOutput").ap()
    with tile.TileContext(nc) as tc:
        with ExitStack() as ctx:
            _dice_body(ctx, tc, out, x, m)
    nc.compile()
    _CACHE["nc"] = nc
    return nc


def _in_maps(output: np.ndarray, masks: np.ndarray):
    output = np.ascontiguousarray(output, dtype=np.float32)
    masks = np.ascontiguousarray(masks, dtype=np.float32)
    return [
        {"x": output[b].reshape(P, COLS), "m": masks[b].reshape(P, COLS)}
        for b in range(N_CORES)
    ]


def _finish(cs: np.ndarray) -> np.float32:
    """Per-subject scalar tail (fp32, mirrors the reference ordering).

    cs: [C, 3] device output — columns (inter, mask_sum, x_sum) per class.
    """
    cs = cs.astype(np.float32)
    inter, msum, xsum = cs[:, 0], cs[:, 1], cs[:, 2]
    w = np.float32(1.0) / (msum * msum + np.float32(EPS))
    total = xsum + msum
    nom = (w * inter).sum(dtype=np.float32)
    den = (w * total + np.float32(EPS)).sum(dtype=np.float32)
    return np.float32(1.0) - np.float32(2.0) * nom / den


def run_sharded(output: np.ndarray, masks: np.ndarray, **spmd_kwargs):
    """Run the SPMD kernel; returns (loss[1], BassKernelResults)."""
    nc = _build()
    res = run_bass_kernel_spmd(
        nc, _in_maps(output, masks), list(range(N_CORES)), **spmd_kwargs
    )
    per_subj = np.array(
        [_finish(res.results[b]["class_sums"]) for b in range(N_CORES)],
        dtype=np.float32,
    )
    loss = (per_subj.sum(dtype=np.float32) / np.float32(B)).reshape(1)
    return loss.astype(np.float32), res


def kernel(output: np.ndarray, masks: np.ndarray) -> np.ndarray:
    loss, _ = run_sharded(output, masks)
    return loss
